# revision 36
# baseline (speedup 1.0000x reference)
"""Trainium2 Bass kernel for nn_Mixture_Loss_74053826118054.

Strategy (pure data parallel: batch axis B=256 sharded over 8 cores):
  Every term of the loss depends only on 5 per-(s,b)-row reductions over D:
    ll = sum_d l^2,  tt = sum_d t^2,  lt = sum_d l*t,
    ln = sum_d l[s]*l[s+1]  (consecutive sentences, same batch),
    tn = sum_d t[s]*t[s+1]
  Each core computes those row arrays for its 32 batches; the tiny O(S*B)
  finish (cos, deltas, rank-compaction, delta-of-delta) runs on host in
  float64, reproducing the reference semantics exactly.

Device layout: rows are batch-major (b, s). Each SBUF partition holds a
window of 17 consecutive rows (16 + 1 overlap), so consecutive-row products
are free-axis slices (partition shifts are illegal on compute engines).
l and t are stacked into one DRAM tensor and each 1024-wide chunk (row slot
j of all 128 windows, both halves) is fetched with a single strided DMA.

Engine assignment (v2): profiling the v1 kernel showed DVE's fused
scalar_tensor_tensor runs at ~1213 ns when GpSimd is idle but ~3352 ns
while GpSimd tensor_tensor traffic hits SBUF (2.8x port contention), while
ACT activations are contention-immune at 1131+278 ns. So v2 bans GpSimd:
  ACT: ll, tt squares with fused accumulate        (32 ops, ~45 us busy)
  DVE: ln, tn, lt as fused stt product+accumulate  (48 ops, ~62 us busy)
DMA (17.4 MB/core) streams underneath at ~46 us. No drains: the final
compute op of each engine carries the done increment (sem updates fire
after the read-accumulator aux op per the HW model).

v2.1 schedule refinements (from the v2 trace):
  - The first stt could only start at 18.8 us (DMA boot + issue + first two
    1 MB chunks). Chunks 0/1 are now fetched as d-halves (x0a,x1a,x0b,x1b)
    and chunk 0's reductions run as half-ops with separate accumulators
    (summed on host), so DVE starts ~6 us earlier.
  - The overlap chunk (16) is issued 6th instead of 17th: as the 17th ring
    entry it was observed to deliver 27 us after its predecessors.
  - Result columns are interleaved per chunk ([5 quants] x 16 chunks + 5
    half-spares) so the output can be shipped as two contiguous DMAs: cols
    0:70 as soon as both engines pass chunk 13, the rest at the end.
"""

import numpy as np

from contextlib import ExitStack

import concourse.bass as bass
import concourse.mybir as mybir
from concourse.bass_utils import run_bass_kernel_spmd

F32 = mybir.dt.float32
AF = mybir.ActivationFunctionType
ALU = mybir.AluOpType

N_CORES = 8
S, B, D = 64, 256, 1024
B_SHARD = B // N_CORES          # 32 batches per core
ROWS = B_SHARD * S              # 2048 real rows per core
G = 16                          # rows per window
P = 128                         # partitions per tile
NMEGA = ROWS // (G * P)         # 1 window-set per core
ROWS_PAD = (P * NMEGA + 1) * G  # one extra window of padding rows
NCOL = NMEGA * G                # 16 result columns
QUANTS = ("ll", "tt", "lt", "ln", "tn")

_cached_nc = None


def _build_program():
    global _cached_nc
    if _cached_nc is not None:
        return _cached_nc
    nc = bass.Bass()
    # row-interleaved: x[r, 0] = l-row r, x[r, 1] = t-row r, so each
    # partition-line of a chunk DMA is one 8 KB contiguous read
    x_in = nc.dram_tensor("x", [ROWS_PAD, 2, D], F32, kind="ExternalInput")
    res_out = nc.dram_tensor("res", [P, 5 * NCOL + 9], F32,
                             kind="ExternalOutput")
    x_v = x_in.rearrange("(w g) v d -> w g v d", g=G)

    with ExitStack() as stack:
        ec = stack.enter_context
        csem = [ec(nc.semaphore(f"c{j}")) for j in range(G + 1)]
        xbig = ec(nc.sbuf_tensor([P, (G + 1) * 2 * D], F32))
        dummies = ec(nc.sbuf_tensor([P, 8], F32))
        res = ec(nc.sbuf_tensor([P, 5 * NCOL + 9], F32))
        ha_sem = ec(nc.semaphore("ha_sem"))
        part_sem = ec(nc.semaphore("part_sem"))
        done_sem = ec(nc.semaphore("done_sem"))
        out_sem = ec(nc.semaphore("out_sem"))
        junk = ec(nc.sbuf_tensor([P, 5 * D], F32))
        block = ec(nc.Block(no_gpsimd_drain=True))
        # result columns interleaved per chunk: col = 5*j + quant_index,
        # then 5 spare columns (80..84) for chunk 0's B-half accumulators
        qidx = {q: i for i, q in enumerate(QUANTS)}
        xc = xbig.ap().rearrange("p (c v d) -> p c v d", v=2, d=D)

        def chunk(j, half, dslc=slice(None)):
            return xc[:, j, half, dslc]

        def rcol(q, j):
            k = 5 * j + qidx[q]
            return res.ap()[:, k:k + 1]

        def scol(k):
            return res.ap()[:, 5 * NCOL + k:5 * NCOL + k + 1]

        def bcast(k, n=D):
            # a real (non-broadcast) scratch output: measured ~6% faster
            # stt than a stride-0 broadcast dummy
            return junk.ap()[:, k * D:k * D + n]

        def semof(j):
            # chunks 13..15 arrive as one DMA job on csem[13]
            return csem[j] if (j < 13 or j == G) else csem[13]

        HA = slice(0, D // 2)
        HB = slice(D // 2, D)

        @block.sync
        def _(sync):
            # chunk 0/1 d-halves first, then chunk 2, the overlap chunk,
            # chunks 3..12, and 13..15 as one job (late chunks have slack;
            # >19 total DMA jobs was observed to slow every DVE op by 20%)
            sync.dma_start(out=xc[:, 0, :, HA],
                           in_=x_v[0:P, 0, :, HA]).then_inc(ha_sem, 16)
            sync.dma_start(out=xc[:, 1, :, HA],
                           in_=x_v[0:P, 1, :, HA]).then_inc(ha_sem, 16)
            sync.dma_start(out=xc[:, 0, :, HB],
                           in_=x_v[0:P, 0, :, HB]).then_inc(csem[0], 16)
            sync.dma_start(out=xc[:, 1, :, HB],
                           in_=x_v[0:P, 1, :, HB]).then_inc(csem[1], 16)
            sync.dma_start(out=xc[:, 2, :, :],
                           in_=x_v[0:P, 2, :, :]).then_inc(csem[2], 16)
            sync.dma_start(out=xc[:, 3, :, :],
                           in_=x_v[0:P, 3, :, :]).then_inc(csem[3], 16)
            sync.dma_start(out=xc[:, 4, :, :],
                           in_=x_v[0:P, 4, :, :]).then_inc(csem[4], 16)
            sync.dma_start(out=xc[:, G, :, :],
                           in_=x_v[1:P + 1, 0, :, :]).then_inc(csem[G], 16)
            for j in range(5, 13):
                sync.dma_start(out=xc[:, j, :, :],
                               in_=x_v[0:P, j, :, :]).then_inc(csem[j], 16)
            sync.dma_start(out=xc[:, 13:16, :, :],
                           in_=x_v[0:P, 13:16, :, :]).then_inc(csem[13], 16)
            # cols 0:75 (all chunks 0..14) once both engines pass j=14
            sync.wait_ge(part_sem, 2)
            sync.dma_start(out=res_out[:, 0:75],
                           in_=res.ap()[:, 0:75]).then_inc(out_sem, 16)
            sync.wait_ge(done_sem, 2)
            sync.dma_start(out=res_out[:, 75:89],
                           in_=res.ap()[:, 75:89]).then_inc(out_sem, 16)
            sync.wait_ge(out_sem, 32)

        @block.scalar
        def _(scalar):
            # chunk 0 as d-halves (A accumulates into the j=0 column, B
            # into a spare column; host adds them)
            scalar.wait_ge(ha_sem, 16)
            scalar.activation(bcast(0, D // 2), chunk(0, 0, HA), AF.Square,
                              accum_out=rcol("ll", 0))
            scalar.activation(bcast(1, D // 2), chunk(0, 1, HA), AF.Square,
                              accum_out=rcol("tt", 0))
            scalar.wait_ge(csem[0], 16)
            scalar.activation(bcast(0, D // 2), chunk(0, 0, HB), AF.Square,
                              accum_out=scol(qidx["ll"]))
            scalar.activation(bcast(1, D // 2), chunk(0, 1, HB), AF.Square,
                              accum_out=scol(qidx["tt"]))
            scalar.wait_ge(ha_sem, 32)
            for j in range(1, G):
                if j < 14:
                    scalar.wait_ge(semof(j), 16)
                scalar.activation(bcast(0), chunk(j, 0), AF.Square,
                                  accum_out=rcol("ll", j))
                ins = scalar.activation(bcast(1), chunk(j, 1), AF.Square,
                                        accum_out=rcol("tt", j))
                if j == 14:
                    ins.then_inc(part_sem, 1)
            ins.then_inc(done_sem, 1)

        @block.vector
        def _(vector):
            # chunk 0 (and its ln/tn partner chunk 1) as d-halves; all
            # A-half ops first (the A-half DMAs are issued first)
            vector.wait_ge(ha_sem, 16)
            vector.scalar_tensor_tensor(
                out=bcast(2, D // 2), in0=chunk(0, 0, HA), scalar=0.0,
                in1=chunk(0, 1, HA), op0=ALU.bypass, op1=ALU.mult,
                accum_out=rcol("lt", 0))
            vector.wait_ge(ha_sem, 32)
            vector.scalar_tensor_tensor(
                out=bcast(3, D // 2), in0=chunk(0, 0, HA), scalar=0.0,
                in1=chunk(1, 0, HA), op0=ALU.bypass, op1=ALU.mult,
                accum_out=rcol("ln", 0))
            vector.scalar_tensor_tensor(
                out=bcast(4, D // 2), in0=chunk(0, 1, HA), scalar=0.0,
                in1=chunk(1, 1, HA), op0=ALU.bypass, op1=ALU.mult,
                accum_out=rcol("tn", 0))
            vector.wait_ge(csem[0], 16)
            vector.scalar_tensor_tensor(
                out=bcast(2, D // 2), in0=chunk(0, 0, HB), scalar=0.0,
                in1=chunk(0, 1, HB), op0=ALU.bypass, op1=ALU.mult,
                accum_out=scol(qidx["lt"]))
            vector.wait_ge(csem[1], 16)
            vector.scalar_tensor_tensor(
                out=bcast(3, D // 2), in0=chunk(0, 0, HB), scalar=0.0,
                in1=chunk(1, 0, HB), op0=ALU.bypass, op1=ALU.mult,
                accum_out=scol(qidx["ln"]))
            vector.scalar_tensor_tensor(
                out=bcast(4, D // 2), in0=chunk(0, 1, HB), scalar=0.0,
                in1=chunk(1, 1, HB), op0=ALU.bypass, op1=ALU.mult,
                accum_out=scol(qidx["tn"]))
            # lt_j needs only chunk j, so run lt one chunk ahead of
            # ln/tn: each arrival sem unlocks {lt_{j+1}, ln_j, tn_j},
            # which fills DVE during the early DMA ramp
            vector.scalar_tensor_tensor(
                out=bcast(2), in0=chunk(1, 0), scalar=0.0,
                in1=chunk(1, 1), op0=ALU.bypass, op1=ALU.mult,
                accum_out=rcol("lt", 1))
            for j in range(1, G):
                vector.wait_ge(semof(j + 1), 16)
                if j < G - 1:
                    vector.scalar_tensor_tensor(
                        out=bcast(2), in0=chunk(j + 1, 0), scalar=0.0,
                        in1=chunk(j + 1, 1), op0=ALU.bypass, op1=ALU.mult,
                        accum_out=rcol("lt", j + 1))
                vector.scalar_tensor_tensor(
                    out=bcast(3), in0=chunk(j, 0), scalar=0.0,
                    in1=chunk(j + 1, 0), op0=ALU.bypass, op1=ALU.mult,
                    accum_out=rcol("ln", j))
                ins = vector.scalar_tensor_tensor(
                    out=bcast(4), in0=chunk(j, 1), scalar=0.0,
                    in1=chunk(j + 1, 1), op0=ALU.bypass, op1=ALU.mult,
                    accum_out=rcol("tn", j))
                if j == 14:
                    ins.then_inc(part_sem, 1)
            ins.then_inc(done_sem, 1)

    _cached_nc = nc
    return nc


def _unpack(arr):
    """(128, NCOL) device layout -> (B_SHARD, S): row r = p*G + j."""
    return arr.reshape(ROWS).reshape(B_SHARD, S)


def _run_device(logits, tgt_out, trace=False):
    """Returns dict q -> (B, S) float32 row-dot arrays, plus kernel results."""
    nc = _build_program()
    # (S, B, D) -> (B, S, D) batch-major, split over cores along B
    lb = np.ascontiguousarray(np.swapaxes(logits, 0, 1))
    tb = np.ascontiguousarray(np.swapaxes(tgt_out, 0, 1))
    in_maps = []
    for c in range(N_CORES):
        sl = slice(c * B_SHARD, (c + 1) * B_SHARD)
        x = np.zeros((ROWS_PAD, 2, D), np.float32)
        x[:ROWS, 0] = lb[sl].reshape(ROWS, D)
        x[:ROWS, 1] = tb[sl].reshape(ROWS, D)
        in_maps.append({"x": x})
    kres = run_bass_kernel_spmd(nc, in_maps, list(range(N_CORES)), trace=trace)
    full = {}
    for i, q in enumerate(QUANTS):
        parts = []
        for c in range(N_CORES):
            r = kres.results[c]["res"]
            arr = r[:, :5 * NCOL].reshape(P, NCOL, 5)[:, :, i].copy()
            arr[:, 0] += r[:, 5 * NCOL + i]      # chunk 0's B-half
            parts.append(_unpack(arr))
        full[q] = np.concatenate(parts, axis=0)
    return full, kres


def _finish_host(rows, mask):
    """Host-side float64 finish: reproduce reference semantics exactly."""
    ll = rows["ll"].astype(np.float64)
    tt = rows["tt"].astype(np.float64)
    lt = rows["lt"].astype(np.float64)
    ln = rows["ln"].astype(np.float64)
    tn = rows["tn"].astype(np.float64)

    valid = ~mask                     # (B, S)
    n_valid = float(valid.sum())

    # masked MSE: sum over valid rows of sum_d (l-t)^2 = ll - 2lt + tt
    mse = ((ll - 2.0 * lt + tt) * valid).sum() / (n_valid * D)

    # CosineEmbeddingLoss part (eps = 1e-8)
    na = np.maximum(np.sqrt(ll), 1e-8)
    nb = np.maximum(np.sqrt(tt), 1e-8)
    c = lt / (na * nb)
    loss_cos = ((1.0 - c) * valid).sum() / n_valid

    # consecutive-sentence cosine deltas (eps = 1e-6), shape (B, S-1)
    nl = np.maximum(np.sqrt(ll), 1e-6)
    nt = np.maximum(np.sqrt(tt), 1e-6)
    d_l = ln[:, :S - 1] / (nl[:, :-1] * nl[:, 1:])
    d_t = tn[:, :S - 1] / (nt[:, :-1] * nt[:, 1:])
    pair_valid = valid[:, :-1] & valid[:, 1:]
    cnt = int(pair_valid.sum())
    loss_delta = (np.square(d_l - d_t) * pair_valid).sum() / max(cnt, 1)

    # delta-of-delta on the compacted (valid-only, batch-major) delta lists
    L = B * (S - 1)
    pvf = pair_valid.reshape(-1)

    def dd(d_flat):
        dense = np.zeros(L, np.float64)
        dense[:cnt] = d_flat[pvf]
        prev = dense[:-1]
        den = np.where(prev != 0, prev, 1e-6)
        return (dense[1:] - prev) / den

    dd_l = dd(d_l.reshape(-1))
    dd_t = dd(d_t.reshape(-1))
    dd_valid = np.arange(L - 1) < (cnt - 1)
    n_dd = float(max(cnt - 1, 1))
    loss_dd = (np.square(dd_l - dd_t) * dd_valid).sum() / n_dd / 100.0

    return mse + loss_cos + loss_delta + loss_dd


def kernel(logits, tgt_out, tgt_padding_mask, _trace=False):
    logits = np.asarray(logits, dtype=np.float32)
    tgt_out = np.asarray(tgt_out, dtype=np.float32)
    mask = np.asarray(tgt_padding_mask).astype(bool)
    rows, kres = _run_device(logits, tgt_out, trace=_trace)
    total = _finish_host(rows, mask)
    out = np.array(total, dtype=np.float32)
    if _trace:
        return out, kres
    return out
